# revision 5
# baseline (speedup 1.0000x reference)
"""v8: v1's proven structure (f16 weights [row,k,plane,col], edge-padded
latent with rs/rso row-shift tiles, DVE f16 2x multiplies) but accumulation
moved OFF the DVE onto the PE: every tap's product goes through 3 identity
matmuls (one per 512-col PSUM bank) accumulating in f32 PSUM.  DVE does only
the 49 multiplies per block (half its former work).  Weight loads batched
per tap-group.  PSUM drains via ScalarE to f16; host upcasts.
"""

import numpy as np

B, C, H, W = 2, 24, 256, 256
R = 7
PAD = R // 2
NCORES = 8
PLANES = B * C
PPC = PLANES // NCORES  # 6
HP = H + 2 * PAD
WP = W + 2 * PAD
BLK = 128
FREE = PPC * W  # 1536
BANK = 512
NBANK = FREE // BANK  # 3
PLPB = BANK // W  # 2
KG = 10  # taps per weight-group DMA
SPLIT_WG = True  # per-tap weight DMAs won on device (-12% vs batched)
USE_RSO = False  # dropping rso copies measured 13% faster

_cache = {}


def _split_multi_waits(nc, max_waits: int = 1):
    import concourse.mybir as mybir

    cnt = 0
    for f in nc.m.functions:
        for b in f.blocks:
            changed = False
            new_insts = []
            for inst in b.instructions:
                si = inst.sync_info
                if si is not None and len(si.on_wait) > max_waits:
                    waits = list(si.on_wait)
                    upds = list(si.on_update)
                    chunks = [
                        waits[i : i + max_waits]
                        for i in range(0, len(waits), max_waits)
                    ]
                    for chunk in chunks[:-1]:
                        nop = mybir.InstNoOp(name=f"ws_nop_{cnt}", ins=[], outs=[])
                        cnt += 1
                        nop.engine = inst.engine
                        nop.sync_info = mybir.SyncInfo(on_wait=chunk, on_update=[])
                        new_insts.append(nop)
                    inst.sync_info = mybir.SyncInfo(on_wait=chunks[-1], on_update=upds)
                    changed = True
                new_insts.append(inst)
            if changed:
                b.instructions = new_insts


def build_nc(loop_reps: int | None = None, reps: int = 1,
             kg: int = KG, split_wg: bool = SPLIT_WG, use_rso: bool = USE_RSO):
    import concourse.bass as bass
    import concourse.mybir as mybir
    from concourse.tile import TileContext

    f16 = mybir.dt.float16
    f32 = mybir.dt.float32

    nc = bass.Bass("TRN2", target_bir_lowering=False, debug=False, num_devices=NCORES)
    u8 = mybir.dt.uint8
    nf = len([k for k in range(R * R) if k % 3 != 0])
    nu = R * R - nf
    wtf = nc.dram_tensor("wtf", [H, nf, PPC, W], f16, kind="ExternalInput").ap()
    wtu = nc.dram_tensor("wtu", [H, nu, PPC, W], u8, kind="ExternalInput").ap()
    fidx = {}
    uidx = {}
    for k in range(R * R):
        if k % 3 == 0:
            uidx[k] = len(uidx)
        else:
            fidx[k] = len(fidx)
    lp = nc.dram_tensor("lp", [PPC, HP, WP], f16, kind="ExternalInput").ap()
    ident = nc.dram_tensor("ident", [BLK, BLK], f16, kind="ExternalInput").ap()
    out = nc.dram_tensor("out", [PPC, H, W], f16, kind="ExternalOutput").ap()

    lp_r = lp.rearrange("pl r d -> r pl d")
    out_r = out.rearrange("pl r c -> r pl c")

    with TileContext(nc) as tc:
        with tc.tile_pool(name="const", bufs=1) as cpool, \
             tc.tile_pool(name="pool", bufs=1) as pool, \
             tc.tile_pool(name="psum", bufs=1, space="PSUM") as psum:
            idt = cpool.tile([BLK, BLK], f16, name="idt")
            nc.sync.dma_start(out=idt[:], in_=ident)

            def rep_body(rep):
                for blk in range(H // BLK):
                    r0 = blk * BLK
                    ps = [
                        psum.tile(
                            [BLK, BANK], f32, name=f"ps_{rep}_{blk}_{nb}",
                            tag=f"ps_{blk}_{nb}", bufs=1,
                        )
                        for nb in range(NBANK)
                    ]
                    nmm = [0] * NBANK
                    rs = {}
                    rso = {}

                    def need_row(i):
                        if i in rs:
                            return
                        t = pool.tile(
                            [BLK, PPC, WP], f16,
                            name=f"rs_{rep}_{blk}_{i}", tag=f"rs{i}", bufs=2,
                        )
                        nc.sync.dma_start(out=t[:], in_=lp_r[r0 + i : r0 + i + BLK])
                        rs[i] = t
                        if use_rso:
                            to = pool.tile(
                                [BLK, PPC, WP], f16,
                                name=f"rso_{rep}_{blk}_{i}", tag=f"rso{i}", bufs=2,
                            )
                            nc.sync.dma_start(
                                out=to[:, :, 0 : WP - 1],
                                in_=lp_r[r0 + i : r0 + i + BLK, :, 1:WP],
                            )
                            rso[i] = to

                    n_groups = (R * R + kg - 1) // kg
                    for g in range(n_groups):
                        g0, g1 = g * kg, min(g * kg + kg, R * R)
                        for i in sorted({k // R for k in range(g0, g1)}):
                            need_row(i)
                        gf = [k for k in range(g0, g1) if k in fidx]
                        gu = [k for k in range(g0, g1) if k in uidx]
                        wgf = pool.tile(
                            [BLK, len(gf), PPC, W], f16,
                            name=f"wgf_{rep}_{blk}_{g0}", tag="wgf", bufs=2,
                        )
                        for n, k in enumerate(gf):
                            nc.sync.dma_start(
                                out=wgf[:, n], in_=wtf[r0 : r0 + BLK, fidx[k]]
                            )
                        wgu = pool.tile(
                            [BLK, max(len(gu), 1), PPC, W], u8,
                            name=f"wgu_{rep}_{blk}_{g0}", tag="wgu", bufs=2,
                        )
                        for n, k in enumerate(gu):
                            nc.sync.dma_start(
                                out=wgu[:, n], in_=wtu[r0 : r0 + BLK, uidx[k]]
                            )
                        for k in range(g0, g1):
                            i, j = divmod(k, R)
                            if use_rso and (j % 2 == 1):
                                x = rso[i][:, :, j - 1 : j - 1 + W]
                            else:
                                x = rs[i][:, :, j : j + W]
                            prod = pool.tile(
                                [BLK, PPC, W], f16,
                                name=f"prod_{rep}_{blk}_{k}", tag="prod", bufs=4,
                            )
                            if k in fidx:
                                w_op = wgf[:, gf.index(k)]
                            else:
                                w_op = wgu[:, gu.index(k)]
                            nc.vector.tensor_mul(prod[:], w_op, x)
                            for nb in range(NBANK):
                                nmm[nb] += 1
                                nc.tensor.matmul(
                                    ps[nb][:],
                                    idt[:],
                                    prod[:, nb * PLPB : (nb + 1) * PLPB, :],
                                    start=(nmm[nb] == 1),
                                    stop=(nmm[nb] == R * R),
                                    skip_group_check=True,
                                )
                    o = pool.tile(
                        [BLK, PPC, W], f16, name=f"o_{rep}_{blk}",
                        tag=f"o{blk}", bufs=2,
                    )
                    for nb in range(NBANK):
                        nc.scalar.copy(o[:, nb * PLPB : (nb + 1) * PLPB, :], ps[nb][:])
                    nc.sync.dma_start(out=out_r[r0 : r0 + BLK], in_=o[:])

            if loop_reps is not None:
                with tc.For_i(0, loop_reps, 1):
                    rep_body(0)
            else:
                for rep in range(reps):
                    rep_body(rep)
    _split_multi_waits(nc)
    return nc


def _prep_inputs(latent, weights):
    lat = np.asarray(latent, dtype=np.float32).reshape(PLANES, H, W)
    wts = np.asarray(weights, dtype=np.float32).reshape(PLANES, R * R, H, W)
    lpad = np.pad(lat / 256.0, ((0, 0), (PAD, PAD), (PAD, PAD)),
                  mode="edge").astype(np.float16)
    fks = [k for k in range(R * R) if k % 3 != 0]
    uks = [k for k in range(R * R) if k % 3 == 0]
    ident = np.eye(BLK, dtype=np.float16)
    in_maps = []
    for c in range(NCORES):
        wcr = wts[c * PPC : (c + 1) * PPC]
        wf = np.ascontiguousarray(
            (wcr[:, fks] * 256.0).transpose(2, 1, 0, 3).astype(np.float16)
        )
        wu = np.ascontiguousarray(
            np.rint(wcr[:, uks] * 256.0).clip(0, 255)
            .transpose(2, 1, 0, 3).astype(np.uint8)
        )
        in_maps.append(
            {
                "wtf": wf,
                "wtu": wu,
                "lp": np.ascontiguousarray(lpad[c * PPC : (c + 1) * PPC]),
                "ident": ident,
            }
        )
    return in_maps


def _get_runner():
    if "runner" in _cache:
        return _cache["runner"]

    import jax
    import concourse.mybir as mybir
    from concourse import bass2jax
    from jax.experimental.shard_map import shard_map
    from jax.sharding import Mesh, NamedSharding, PartitionSpec

    bass2jax.install_neuronx_cc_hook()
    nc = build_nc(reps=1)

    partition_name = nc.partition_id_tensor.name if nc.partition_id_tensor else None
    in_names, out_names, out_avals, zero_outs = [], [], [], []
    for alloc in nc.m.functions[0].allocations:
        if not isinstance(alloc, mybir.MemoryLocationSet):
            continue
        name = alloc.memorylocations[0].name
        if alloc.kind == "ExternalInput":
            if name != partition_name:
                in_names.append(name)
        elif alloc.kind == "ExternalOutput":
            out_names.append(name)
            shape = tuple(alloc.tensor_shape)
            dtype = mybir.dt.np(alloc.dtype)
            out_avals.append(jax.core.ShapedArray(shape, dtype))
            zero_outs.append(np.zeros(shape, dtype))
    n_params = len(in_names)
    all_in_names = list(in_names) + out_names
    if partition_name is not None:
        all_in_names.append(partition_name)

    def _body(*args):
        operands = list(args)
        if partition_name is not None:
            operands.append(bass2jax.partition_id_tensor())
        return tuple(
            bass2jax._bass_exec_p.bind(
                *operands,
                out_avals=tuple(out_avals),
                in_names=tuple(all_in_names),
                out_names=tuple(out_names),
                lowering_input_output_aliases=(),
                sim_require_finite=True,
                sim_require_nnan=True,
                nc=nc,
            )
        )

    devices = jax.devices()[:NCORES]
    mesh = Mesh(np.asarray(devices), ("core",))
    in_specs = (PartitionSpec("core"),) * (n_params + len(out_names))
    out_specs = (PartitionSpec("core"),) * len(out_names)
    sharded = jax.jit(
        shard_map(
            _body, mesh=mesh, in_specs=in_specs, out_specs=out_specs, check_rep=False
        ),
        keep_unused=True,
    )
    sh = NamedSharding(mesh, PartitionSpec("core"))
    zeros_dev = [
        jax.device_put(np.zeros((NCORES * z.shape[0], *z.shape[1:]), z.dtype), sh)
        for z in zero_outs
    ]

    def run(in_maps):
        ins_dev = [
            jax.device_put(
                np.concatenate([in_maps[c][n] for c in range(NCORES)], axis=0), sh
            )
            for n in in_names
        ]
        outs = sharded(*ins_dev, *zeros_dev)
        jax.block_until_ready(outs)
        return np.asarray(outs[0])

    _cache["runner"] = run
    return run


def kernel(latent, weights, window_size):
    r = int(window_size)
    assert r == R, f"kernel hardcoded for window_size={R}, got {r}"
    run = _get_runner()
    in_maps = _prep_inputs(latent, weights)
    full = run(in_maps)
    return full.reshape(B, C, H, W).astype(np.float32)
